# revision 45
# baseline (speedup 1.0000x reference)
"""Trainium2 Bass kernel for a ViT-Base transformer encoder block.

Input x: [64, 197, 768] fp32 + weights. Data-parallel over batch across 8
NeuronCores (8 batches/core = 1576 tokens/core). All matmul operands are
fp16 (fp32 PSUM accumulation): same PE throughput as bf16 but 8x finer
mantissa, so quantization error stays ~4.5e-4. Weights and x are cast to
fp16 host-side, and x is additionally passed pre-transposed (d-major) so no
PE transposes are needed in pass 1.

Per core, two passes over 4 batch-pairs (2 batches = 394 tokens each):

  pass 1: QKV projections, software-pipelined attention (per-batch
          197-col matmuls; odd heads write PSUM partitions 64:128 via
          tile_position; softmax denominators via rowsum matmuls +
          reciprocal_approx_fast + gpsimd partition_broadcast, two heads
          packed per broadcast tile so ctx eviction runs full-width),
          O-projection, LayerNorm1 + residual -> x1 kept in SBUF (fp16).
          Pair p+1's projections are interleaved into pair p's attention
          pipeline to keep the in-order PE queue dense.
  pass 2: MLP with W1/W2 resident in SBUF (fp16), exact GELU fused into
          the PSUM eviction, PE transpose back to token-major,
          LayerNorm2 + residual -> out.

LayerNorm scale/shift application runs on the vector engine (tensor_scalar
with per-partition rstd/-mu*rstd) so the scalar engine only ever runs Exp
(pass 1) / Gelu (pass 2) plus the LN Sqrt, minimizing activation-table
reloads. When gamma==1 / beta==0 (true for this problem's inputs, checked
at build time) the affine ops are folded into the residual add.
"""
import os
import sys

sys.path.insert(0, "/opt/trn_rl_repo")

import numpy as np
import ml_dtypes
from contextlib import ExitStack

import concourse.bass as bass
import concourse.tile as tile
from concourse import bacc, mybir
from concourse.bass_utils import run_bass_kernel_spmd
from concourse.masks import make_identity

DIM, NH, HD, HID = 768, 12, 64, 3072
S = 197
B = 64
N_CORES = 8
BPC = B // N_CORES            # 8 batches per core
T = BPC * S                   # 1576 tokens per core
NPAIR = BPC // 2              # 4 batch pairs per core
PT = 2 * S                    # 394 tokens per pair
PTP = 400                     # PT padded to a 16-elem multiple
EPS = 1e-6
DC = DIM // 128               # 6 d-chunks
HC = HID // 128               # 24 hidden chunks

F32 = mybir.dt.float32
F16 = mybir.dt.float16
FP8 = mybir.dt.float8e4
AF = mybir.ActivationFunctionType
OP = mybir.AluOpType
DR = mybir.MatmulPerfMode.DoubleRow

# Partial-fp8 MLP1: contract the first NC8 of 6 k-chunks in fp8 DoubleRow
# (2 chunks per PE instruction). Error budget: measured 5.4e-4 all-fp16;
# 4 fp8 chunks add ~1.8e-2 (sim), still under the 2e-2 gate.
NC8 = int(os.environ.get("BASSK_FP8_MLP1", "2"))
assert NC8 in (0, 2, 4)
W1S = 8.0 if NC8 else 1.0     # W1 pre-scale so fp8 chunks stay normal-range

# token tiles within a pair: (offset, size); tile 2*b + s for batch b
Q_TILES = [(0, 128), (128, 69), (197, 128), (325, 69)]

DEBUG = bool(int(os.environ.get("BASSK_DEBUG", "0")))

_cached = None


def _build(ln1_trivial, ln2_trivial):
    nc = bacc.Bacc("TRN2", target_bir_lowering=False, debug=False)

    # host-side pre-shuffled layouts: weights [128, C*out] fp16, xt
    # [128, NPAIR*DC*PTP] fp16 d-major, x [T, DIM] fp16 (residual path),
    # broadcast biases pre-tiled [128, 7*DIM], per-partition biases [128, 36]
    x_d = nc.dram_tensor("x", [T, DIM], F16, kind="ExternalInput").ap()
    xt_d = nc.dram_tensor("xt", [128, NPAIR * DC * PTP], F16,
                          kind="ExternalInput").ap()
    w_d = {}
    w1_specs = ([("W1a", [128, NC8 * HID], FP8),
                 ("W1b", [128, (DC - NC8) * HID], F16)] if NC8 else
                [("W1b", [128, DC * HID], F16)])
    for name, shape, dt in [("Wq", [128, DC * DIM], F16),
                            ("Wk", [128, DC * DIM], F16),
                            ("Wv", [128, DC * DIM], F16),
                            ("Wo", [128, DC * DIM], F16),
                            *w1_specs,
                            ("W2", [128, HC * DIM], F16),
                            ("bcbr", [1, 7 * DIM], F16),
                            ("bpp", [128, 36], F32)]:
        w_d[name] = nc.dram_tensor(name, shape, dt, kind="ExternalInput").ap()
    out_d = nc.dram_tensor("out", [T, DIM], F32, kind="ExternalOutput").ap()

    with tile.TileContext(nc) as tc, ExitStack() as octx:
        persist = octx.enter_context(tc.tile_pool(name="persist", bufs=1))

        # ---- pools for x slices (allocated before any DMA so the
        # startup-critical pair-0 transfers can be issued first) ----
        xt_view = xt_d.rearrange("p (r c t) -> p r c t", c=DC, t=PTP)

        # ---- constants / persistent tiles ----
        ident_f = persist.tile([128, 128], F32)
        ident_b = persist.tile([128, 128], F16)
        ones_b = persist.tile([128, 1], F16)
        eps_sb = persist.tile([128, 1], F32)
        bpp = persist.tile([128, 36], F32)
        bcb = persist.tile([128, 7, DIM], F16)
        Wt = {}
        for name in ["Wq", "Wk", "Wv", "Wo"]:
            Wt[name] = persist.tile([128, DC, DIM], F16, name=f"wt_{name}",
                                    tag=f"wt_{name}")
        W1a = (persist.tile([128, NC8, HID], FP8, name="w1a", tag="w1a")
               if NC8 else None)
        W1b = persist.tile([128, DC - NC8, HID], F16, name="w1b", tag="w1b")
        # x1 = attn block output, kept in SBUF across passes (fp16)
        x1_all = persist.tile([128, NPAIR, 4, DIM], F16)

        bq_sb = bpp[:, 0:DC]
        bk_sb = bpp[:, DC:2 * DC]
        b1_sb = bpp[:, 2 * DC:2 * DC + HC]
        bcast = {name: bcb[:, i, :]
                 for i, name in enumerate(
                     ["bv", "bo", "b2", "g1", "be1", "g2", "be2"])}

        # =========================== PASS 1 ===========================
        with ExitStack() as ctx:
            xtp_pool = ctx.enter_context(tc.tile_pool(name="xtf", bufs=2))
            xp = ctx.enter_context(tc.tile_pool(name="xp", bufs=3))
            big = ctx.enter_context(tc.tile_pool(name="p1big", bufs=2))
            exp_pool = ctx.enter_context(tc.tile_pool(name="exp", bufs=6))
            rt_pool = ctx.enter_context(tc.tile_pool(name="rt", bufs=2))
            bc_pool = ctx.enter_context(tc.tile_pool(name="bc", bufs=3))
            ao_pool = ctx.enter_context(tc.tile_pool(name="ao", bufs=2))
            ln_pool = ctx.enter_context(tc.tile_pool(name="ln", bufs=3))

            ps_mm = ctx.enter_context(tc.tile_pool(name="psmm", bufs=2, space="PSUM"))
            ps_sp = ctx.enter_context(tc.tile_pool(name="pssp", bufs=4, space="PSUM"))
            ps_cx = ctx.enter_context(tc.tile_pool(name="pscx", bufs=2, space="PSUM"))

            def load_xt(p):
                xT = xtp_pool.tile([128, DC, PTP], F16, tag="xT", name="xT")
                nc.sync.dma_start(xT[:], xt_view[:, p, :, :])
                return xT

            def load_x(p):
                g0 = p * PT
                x_sb = xp.tile([128, 4, DIM], F16, tag="x", name="x_sb")
                for i, (off, sz) in enumerate(Q_TILES):
                    nc.sync.dma_start(x_sb[0:sz, i, :], x_d[g0 + off:g0 + off + sz, :])
                return x_sb

            # --- startup-critical DMAs, most-urgent first per queue.
            # The first qk items need only the first 128-col block of Wq/Wk,
            # so those land first; W1 (pass-2) rides last on the gpsimd queue.
            xT_cur = load_xt(0)                      # sync queue
            wv = {n: w_d[n].rearrange("p (c j) -> p c j", j=DIM)
                  for n in ["Wq", "Wk", "Wv", "Wo"]}
            # weight loads spread over the scalar AND gpsimd DMA rings (a
            # single ring is the startup bottleneck at ~130GB/s); broadcast
            # biases come as one [1, 5376] row expanded on the gpsimd engine
            nc.scalar.dma_start(Wt["Wq"][:, :, 0:128], wv["Wq"][:, :, 0:128])
            nc.gpsimd.dma_start(Wt["Wk"][:, :, 0:128], wv["Wk"][:, :, 0:128])
            nc.scalar.dma_start(Wt["Wq"][:, :, 128:DIM],
                                wv["Wq"][:, :, 128:DIM])
            nc.gpsimd.dma_start(Wt["Wk"][:, :, 128:DIM],
                                wv["Wk"][:, :, 128:DIM])
            nc.scalar.dma_start(Wt["Wv"][:], wv["Wv"])
            nc.sync.dma_start(bpp[:], w_d["bpp"])
            stg = ctx.enter_context(tc.tile_pool(name="stg", bufs=1))
            bcbr = stg.tile([128, 7 * DIM], F16, tag="bcbr", name="bcbr")
            nc.sync.dma_start(bcbr[0:1, :], w_d["bcbr"])
            nc.gpsimd.partition_broadcast(
                bcb.rearrange("p a j -> p (a j)"), bcbr[0:1, :], channels=128)
            make_identity(nc, ident_f[:])
            nc.vector.tensor_copy(ident_b[:], ident_f[:])
            nc.vector.memset(ones_b[:], 1.0)
            nc.vector.memset(eps_sb[:], EPS)
            xT_nxt = load_xt(1)
            nc.scalar.dma_start(Wt["Wo"][:], wv["Wo"])
            if NC8:
                nc.gpsimd.dma_start(
                    W1a[:], w_d["W1a"].rearrange("p (c j) -> p c j", j=HID))
            nc.gpsimd.dma_start(
                W1b[:], w_d["W1b"].rearrange("p (c j) -> p c j", j=HID))
            x_sb0 = load_x(0)
            nxt_x = load_x(1)

            # --- projection work items for one pair (list of thunks) ---
            def proj_items(p, xT, dst):
                qT = big.tile([128, DC, PT], F16, tag="qT", name="qT")
                kT = big.tile([128, DC, PT], F16, tag="kT", name="kT")
                v_sb = big.tile([128, 4, NH, HD], F16, tag="v", name="v_sb")
                dst["qT"], dst["kT"], dst["v_sb"] = qT, kT, v_sb
                items = []

                def qk_item(wname, bsb, dstT, c):
                    def run():
                        pm = ps_mm.tile([128, 512], F32, tag="mm", name="pm")
                        for kc in range(DC):
                            nc.tensor.matmul(pm[:, 0:PT],
                                             Wt[wname][:, kc, c * 128:(c + 1) * 128],
                                             xT[:, kc, 0:PT],
                                             start=(kc == 0), stop=(kc == DC - 1))
                        with nc.allow_low_precision(reason="qk f16"):
                            nc.vector.tensor_scalar(dstT[:, c, :], pm[:, 0:PT],
                                                    bsb[:, c:c + 1], None, OP.add)
                    return run

                def v_item(i, s):
                    def run():
                        off, sz = Q_TILES[i]
                        pm = ps_mm.tile([128, 512], F32, tag="mm", name="pm")
                        for kc in range(DC):
                            nc.tensor.matmul(pm[0:sz, 0:384],
                                             xT[:, kc, off:off + sz],
                                             Wt["Wv"][:, kc, s * 384:(s + 1) * 384],
                                             start=(kc == 0), stop=(kc == DC - 1))
                        with nc.allow_low_precision(reason="v f16"):
                            nc.vector.tensor_add(
                                v_sb[0:sz, i, 6 * s:6 * s + 6, :],
                                pm[0:sz, 0:384].rearrange("p (a b) -> p a b", a=6),
                                bcast["bv"][0:sz, s * 384:(s + 1) * 384]
                                    .rearrange("p (a b) -> p a b", a=6))
                    return run

                for c in range(DC):
                    items.append(qk_item("Wq", bq_sb, qT, c))
                    items.append(qk_item("Wk", bk_sb, kT, c))
                for i in range(4):
                    for s in range(2):
                        items.append(v_item(i, s))
                return items

            # --- attention for pair p, interleaving `items` (next pair's
            #     projections) between pipeline steps ---
            def attention(p, cur, items):
                qT, kT, v_sb = cur["qT"], cur["kT"], cur["v_sb"]
                ctxT = big.tile([128, DC, PTP], F16, tag="ctxT", name="ctxT")
                cur["ctxT"] = ctxT
                it = 0
                NSTEP = NH + 2
                exps = {}   # h -> [exp_s0, exp_s1]
                pcs = {}    # hc -> psum tile
                bcs = {}    # hc -> [128, PT] tile, head h in rows (h%2)*64
                for step in range(NSTEP):
                    # stage S: scores + exp for head `step`
                    if step < NH:
                        h = step
                        hc, hp = h // 2, (h % 2) * 64
                        exps[h] = []
                        for s in range(2):
                            ksz = Q_TILES[s][1]
                            psc = ps_sp.tile([128, 512], F32, tag="sp", name="psc")
                            for b in range(2):
                                koff = Q_TILES[2 * b + s][0]
                                cs = slice(b * S, (b + 1) * S)
                                nc.tensor.matmul(
                                    psc[0:ksz, cs],
                                    kT[hp:hp + 64, hc, koff:koff + ksz],
                                    qT[hp:hp + 64, hc, cs],
                                    start=True, stop=True,
                                    skip_group_check=True)
                            et = exp_pool.tile([128, 394], F16, tag="exp", name="et")
                            with nc.allow_low_precision(reason="softmax exp f16"):
                                nc.scalar.activation(et[0:ksz, :], psc[0:ksz, 0:PT],
                                                     AF.Exp, bias=0.0, scale=0.125)
                            exps[h].append(et)
                    # stage R: rowsum + recip + broadcast for head step-1
                    if 1 <= step <= NH:
                        h = step - 1
                        hc, hp = h // 2, (h % 2) * 64
                        pq = ps_sp.tile([128, 512], F32, tag="sp", name="pq")
                        for s in range(2):
                            ksz = Q_TILES[s][1]
                            nc.tensor.matmul(
                                pq[0:1, 0:PT],
                                ones_b[0:ksz, 0:1],
                                exps[h][s][0:ksz, :],
                                start=(s == 0), stop=(s == 1),
                                skip_group_check=True)
                        rt = rt_pool.tile([128, 394], F32, tag="rt", name="rt")
                        with nc.allow_low_precision(reason="softmax recip"):
                            nc.vector.reciprocal_approx_fast(
                                rt[0:1, :], pq[0:1, 0:PT])
                        # partition_broadcast can only target partition 0-based
                        # rows, so each head gets its own full broadcast tile
                        bcs[h] = bc_pool.tile([128, 394], F32, tag="bcsb",
                                              name="bc")
                        nc.gpsimd.partition_broadcast(bcs[h][:, :], rt[0:1, :],
                                                      channels=128)
                    # stage C: ctx for head step-1
                    if 1 <= step <= NH:
                        h = step - 1
                        hc, hp = h // 2, (h % 2) * 64
                        if hp == 0:
                            pcs[hc] = ps_cx.tile([128, 512], F32, tag="cx",
                                                 name=f"cx{hc}")
                        pc = pcs[hc]
                        for b in range(2):
                            cs = slice(b * S, (b + 1) * S)
                            for s in range(2):
                                ksz = Q_TILES[s][1]
                                nc.tensor.matmul(
                                    pc[hp:hp + 64, cs],
                                    v_sb[0:ksz, 2 * b + s, h, :],
                                    exps[h][s][0:ksz, cs],
                                    start=(s == 0), stop=(s == 1),
                                    skip_group_check=True)
                    # interleave next-pair projection work
                    for _ in range(1):
                        if it < len(items):
                            items[it]()
                            it += 1
                    # stage N: normalize head step-2
                    if step >= 2:
                        h2 = step - 2
                        hc, hp = h2 // 2, (h2 % 2) * 64
                        with nc.allow_low_precision(reason="ctx f16"):
                            nc.vector.tensor_tensor(
                                ctxT[hp:hp + 64, hc, 0:PT],
                                pcs[hc][hp:hp + 64, 0:PT],
                                bcs[h2][hp:hp + 64, :], OP.mult)
                return items[it:]

            def _ln_apply(dst, src, mv, eps_ap, gname, bname, trivial, res_ap,
                          sz):
                """LayerNorm scale/shift on the vector engine + residual add.

                src: [sz, DIM] f32 pre-LN values (mean/var in mv).
                dst: fp16 (or f32) destination; res_ap added in.
                """
                sd = ln_pool.tile([128, 2], F32, tag="sd", name="sd")
                nc.scalar.activation(sd[0:sz, 0:1], mv[0:sz, 1:2], AF.Sqrt,
                                     bias=eps_ap[0:sz, :], scale=1.0)
                rstd = ln_pool.tile([128, 1], F32, tag="rstd", name="rstd")
                with nc.allow_low_precision(reason="ln recip"):
                    nc.vector.reciprocal_approx_fast(rstd[0:sz, :], sd[0:sz, 0:1])
                nmr = ln_pool.tile([128, 1], F32, tag="nmr", name="nmr")
                nc.vector.tensor_scalar(nmr[0:sz, :], mv[0:sz, 0:1],
                                        rstd[0:sz, :], -1.0, OP.mult, OP.mult)
                tn = ao_pool.tile([128, DIM], F32, tag="tn", name="tn")
                nc.vector.tensor_scalar(tn[0:sz, :], src[0:sz, :],
                                        rstd[0:sz, :], nmr[0:sz, :],
                                        OP.mult, OP.add)
                with nc.allow_low_precision(reason="ln out"):
                    if trivial:
                        nc.vector.tensor_add(dst, tn[0:sz, :], res_ap)
                    else:
                        nc.vector.tensor_tensor(tn[0:sz, :], tn[0:sz, :],
                                                bcast[gname][0:sz, :], OP.mult)
                        nc.vector.tensor_add(tn[0:sz, :], tn[0:sz, :],
                                             bcast[bname][0:sz, :])
                        nc.vector.tensor_add(dst, tn[0:sz, :], res_ap)

            def o_proj_ln1(p, cur, items):
                it = 0
                ctxT, x_sb = cur["ctxT"], cur["x_sb"]
                for i, (off, sz) in enumerate(Q_TILES):
                    ao = ao_pool.tile([128, DIM], F32, tag="ao", name="ao")
                    for s in range(2):
                        pm = ps_mm.tile([128, 512], F32, tag="mm", name="pm")
                        for c in range(DC):
                            nc.tensor.matmul(pm[0:sz, 0:384],
                                             ctxT[:, c, off:off + sz],
                                             Wt["Wo"][:, c, s * 384:(s + 1) * 384],
                                             start=(c == 0), stop=(c == DC - 1))
                        nc.vector.tensor_add(ao[0:sz, s * 384:(s + 1) * 384],
                                             pm[0:sz, 0:384],
                                             bcast["bo"][0:sz, s * 384:(s + 1) * 384])
                        if it < len(items):
                            items[it]()
                            it += 1
                    # LayerNorm 1 (stats on vector, 2 groups of 384)
                    st = ln_pool.tile([128, 2, nc.vector.BN_STATS_DIM], F32,
                                      tag="st", name="st")
                    for g in range(2):
                        nc.vector.bn_stats(st[0:sz, g, :],
                                           ao[0:sz, g * 384:(g + 1) * 384])
                    mv = ln_pool.tile([128, nc.vector.BN_AGGR_DIM], F32, tag="mv",
                                      name="mv")
                    nc.vector.bn_aggr(mv[0:sz, :], st[0:sz, :, :])
                    _ln_apply(x1_all[0:sz, p, i, :], ao, mv, eps_sb,
                              "g1", "be1", ln1_trivial, x_sb[0:sz, i, :], sz)
                while it < len(items):
                    items[it]()
                    it += 1

            # ---- pass-1 driver: pipelined over pairs ----
            cur = {}
            cur["x_sb"] = x_sb0
            for item in proj_items(0, xT_cur, cur):
                item()
            for p in range(NPAIR):
                if p + 1 < NPAIR:
                    nxt = {"x_sb": nxt_x}
                    items = proj_items(p + 1, xT_nxt, nxt)
                else:
                    nxt = None
                    items = []
                if p + 2 < NPAIR:
                    xT_nxt2 = load_xt(p + 2)
                left = attention(p, cur, items)
                if p + 2 < NPAIR:
                    nxt_x = load_x(p + 2)
                o_proj_ln1(p, cur, left)
                cur = nxt
                if p + 2 < NPAIR:
                    xT_cur, xT_nxt = xT_nxt, xT_nxt2

        # =========================== PASS 2 ===========================
        with ExitStack() as ctx:
            wpool = ctx.enter_context(tc.tile_pool(name="w2p", bufs=1))
            W2t = wpool.tile([128, HC, DIM], F16)
            # split so the first chunks (needed at hcx=LAG) land early
            w2v = w_d["W2"].rearrange("p (c j) -> p c j", j=DIM)
            nc.sync.dma_start(W2t[:, 0:6, :], w2v[:, 0:6, :])
            nc.sync.dma_start(W2t[:, 6:HC, :], w2v[:, 6:HC, :])

            xtp = ctx.enter_context(tc.tile_pool(name="xtp", bufs=2))
            htp = ctx.enter_context(tc.tile_pool(name="htp", bufs=1))
            mo_pool = ctx.enter_context(tc.tile_pool(name="mo", bufs=2))
            moT_pool = ctx.enter_context(tc.tile_pool(name="moT", bufs=2))
            ln_pool = ctx.enter_context(tc.tile_pool(name="ln2", bufs=3))
            ao2_pool = ctx.enter_context(tc.tile_pool(name="ao2", bufs=2))
            out_pool = ctx.enter_context(tc.tile_pool(name="outp", bufs=2))

            ps_wk = ctx.enter_context(tc.tile_pool(name="pswk", bufs=2, space="PSUM"))
            ps_ac = ctx.enter_context(tc.tile_pool(name="psac", bufs=6, space="PSUM"))

            def x1t_items(p, box):
                if NC8:
                    x1T8 = xtp.tile([128, NC8, PTP], FP8, tag="x1T8",
                                    name="x1T8")
                    box["x1T8"] = x1T8
                x1T = xtp.tile([128, DC - NC8, PTP], F16, tag="x1T", name="x1T")
                box["x1T"] = x1T

                def one(i):
                    def run():
                        off, sz = Q_TILES[i]
                        # 6 transposes into one PSUM bank (f16 view), then a
                        # single strided eviction per destination dtype
                        pt = ps_wk.tile([128, 512], F32, tag="wk", name="pt")
                        ptb = pt[:, 0:384].bitcast(F16).rearrange(
                            "p (c t) -> p c t", c=DC)
                        for c in range(DC):
                            nc.tensor.transpose(
                                ptb[:, c, 0:sz],
                                x1_all[0:sz, p, i, c * 128:(c + 1) * 128],
                                ident_b[0:sz, 0:sz])
                        with nc.allow_low_precision(reason="x1T f16/fp8"):
                            if NC8:
                                nc.vector.tensor_copy(x1T8[:, :, off:off + sz],
                                                      ptb[:, 0:NC8, 0:sz])
                            nc.vector.tensor_copy(x1T[:, :, off:off + sz],
                                                  ptb[:, NC8:DC, 0:sz])
                    return run
                return [one(i) for i in range(4)]

            LAG = 6

            def mlp(p, xbox, evict_items, tail_its, mcols=(0, PT)):
                x1T = xbox["x1T"]
                x1T8 = xbox.get("x1T8")
                hT = htp.tile([128, HC, PTP], F16, tag="hT", name="hT")
                pacs = [ps_ac.tile([128, 512], F32, tag="ac", name=f"pac{c}")
                        for c in range(DC)]
                m0, m1 = mcols
                ti = 0
                for hcx in range(HC + LAG):
                    if hcx < HC:
                        pm = ps_wk.tile([128, 512], F32, tag="wk", name="pm")
                        hs = slice(hcx * 128, (hcx + 1) * 128)
                        for kc in range(0, NC8, 2):
                            nc.tensor.matmul(pm[:, 0:PT],
                                             W1a[:, kc:kc + 2, hs],
                                             x1T8[:, kc:kc + 2, 0:PT],
                                             start=(kc == 0), stop=False,
                                             perf_mode=DR,
                                             skip_group_check=True)
                        for j in range(DC - NC8):
                            nc.tensor.matmul(pm[:, 0:PT],
                                             W1b[:, j, hs],
                                             x1T[:, j, 0:PT],
                                             start=(NC8 == 0 and j == 0),
                                             stop=(j == DC - NC8 - 1),
                                             skip_group_check=True)
                        with nc.allow_low_precision(reason="h f16"):
                            nc.scalar.activation(hT[:, hcx, 0:PT], pm[:, 0:PT],
                                                 AF.Gelu,
                                                 bias=b1_sb[:, hcx:hcx + 1],
                                                 scale=1.0 / W1S)
                        if hcx < len(evict_items):
                            evict_items[hcx]()
                    h2 = hcx - LAG
                    if h2 >= 0:
                        for c in range(DC):
                            nc.tensor.matmul(pacs[c][:, m0:m1],
                                             W2t[:, h2, c * 128:(c + 1) * 128],
                                             hT[:, h2, m0:m1],
                                             start=(h2 == 0), stop=(h2 == HC - 1))
                    if hcx >= LAG and hcx % 2 == 0 and ti < len(tail_its):
                        tail_its[ti]()
                        ti += 1
                while ti < len(tail_its):
                    tail_its[ti]()
                    ti += 1
                return pacs, hT

            def evict_items_for(p, pacs, box, cols=(0, PT)):
                if "moT" not in box:
                    box["moT"] = moT_pool.tile([128, DC, PTP], F16, tag="moT",
                                               name="moT")
                moT = box["moT"]
                c0, c1 = cols

                def one(c):
                    def run():
                        with nc.allow_low_precision(reason="moT f16"):
                            nc.vector.tensor_copy(moT[:, c, c0:c1],
                                                  pacs[c][:, c0:c1])
                    return run
                return [one(c) for c in range(DC)]

            def tail_items_for(p, box):
                g0 = p * PT
                moT = box["moT"]

                def one(i):
                    def run():
                        off, sz = Q_TILES[i]
                        _ln2_tile(p, g0, moT, i, off, sz)
                    return run
                return [one(i) for i in range(4)]

            def _ln2_tile(p, g0, moT, i, off, sz):
                mo = mo_pool.tile([128, DIM], F32, tag="mo", name="mo")
                pt = ps_wk.tile([128, 512], F32, tag="wk", name="pt")
                ptb = pt[:, 0:384].bitcast(F16).rearrange(
                    "p (c t) -> p c t", c=DC)
                for c in range(DC):
                    nc.tensor.transpose(ptb[0:sz, c, 0:128],
                                        moT[:, c, off:off + sz], ident_b[:, :])
                nc.vector.tensor_add(
                    mo[0:sz, :],
                    ptb[0:sz, :, :].rearrange("p c t -> p (c t)"),
                    bcast["b2"][0:sz, :])
                # LayerNorm 2 + residual
                st = ln_pool.tile([128, 2, nc.vector.BN_STATS_DIM], F32,
                                  tag="st", name="st")
                for g in range(2):
                    nc.vector.bn_stats(st[0:sz, g, :],
                                       mo[0:sz, g * 384:(g + 1) * 384])
                mv = ln_pool.tile([128, nc.vector.BN_AGGR_DIM], F32, tag="mv",
                                  name="mv")
                nc.vector.bn_aggr(mv[0:sz, :], st[0:sz, :, :])
                sd = ln_pool.tile([128, 2], F32, tag="sd", name="sd")
                nc.scalar.activation(sd[0:sz, 0:1], mv[0:sz, 1:2], AF.Sqrt,
                                     bias=eps_sb[0:sz, :], scale=1.0)
                rstd = ln_pool.tile([128, 1], F32, tag="rstd", name="rstd")
                with nc.allow_low_precision(reason="ln2 recip"):
                    nc.vector.reciprocal_approx_fast(rstd[0:sz, :], sd[0:sz, 0:1])
                nmr = ln_pool.tile([128, 1], F32, tag="nmr", name="nmr")
                nc.vector.tensor_scalar(nmr[0:sz, :], mv[0:sz, 0:1],
                                        rstd[0:sz, :], -1.0, OP.mult, OP.mult)
                tln = ao2_pool.tile([128, DIM], F32, tag="tln", name="tln")
                nc.vector.tensor_scalar(tln[0:sz, :], mo[0:sz, :],
                                        rstd[0:sz, :], nmr[0:sz, :],
                                        OP.mult, OP.add)
                ot = out_pool.tile([128, DIM], F32, tag="ot", name="ot")
                if ln2_trivial:
                    nc.vector.tensor_add(ot[0:sz, :], tln[0:sz, :],
                                         x1_all[0:sz, p, i, :])
                else:
                    nc.vector.tensor_tensor(tln[0:sz, :], tln[0:sz, :],
                                            bcast["g2"][0:sz, :], OP.mult)
                    nc.vector.tensor_add(tln[0:sz, :], tln[0:sz, :],
                                         bcast["be2"][0:sz, :])
                    nc.vector.tensor_add(ot[0:sz, :], tln[0:sz, :],
                                         x1_all[0:sz, p, i, :])
                nc.sync.dma_start(out_d[g0 + off:g0 + off + sz, :], ot[0:sz, :])

            box0 = {}
            for it in x1t_items(0, box0):
                it()
            xbox_cur = box0
            prev_pacs = prev_hT = None
            for p in range(NPAIR):
                ev = []
                tl = []
                if p > 0:
                    pbox = {}
                    ev = evict_items_for(p - 1, prev_pacs, pbox)
                    tl = tail_items_for(p - 1, pbox)
                nbox = {}
                nxt_items = x1t_items(p + 1, nbox) if p + 1 < NPAIR else []
                last = p == NPAIR - 1
                prev_pacs, prev_hT = mlp(p, xbox_cur, ev, tl + nxt_items,
                                         mcols=(0, 128) if last else (0, PT))
                if p + 1 < NPAIR:
                    xbox_cur = nbox
            # final tail: the last pair's MLP2 runs per token tile, each
            # tile's evictions + LN2 hidden under the next tile's matmuls
            fbox = {}
            pend = evict_items_for(NPAIR - 1, prev_pacs, fbox, cols=(0, 128))
            tl_items = tail_items_for(NPAIR - 1, fbox)
            pend = pend + [tl_items[0]]
            for tix in range(1, 4):
                off, sz = Q_TILES[tix]
                ai = 0
                for h2 in range(HC):
                    for c in range(DC):
                        nc.tensor.matmul(prev_pacs[c][:, off:off + sz],
                                         W2t[:, h2, c * 128:(c + 1) * 128],
                                         prev_hT[:, h2, off:off + sz],
                                         start=(h2 == 0), stop=(h2 == HC - 1))
                    if h2 >= 1 and h2 % 3 == 1 and ai < len(pend):
                        pend[ai]()
                        ai += 1
                while ai < len(pend):
                    pend[ai]()
                    ai += 1
                pend = (evict_items_for(NPAIR - 1, prev_pacs, fbox,
                                        cols=(off, off + sz)) +
                        [tl_items[tix]])
            for it in pend:
                it()

    nc.compile()
    return nc


def kernel(x, Wq, bq, Wk, bk, Wv, bv, Wo, bo, W1, b1, W2, b2, g1, be1, g2, be2):
    global _cached
    ln1_trivial = bool(np.all(np.asarray(g1) == 1.0) and
                       np.all(np.asarray(be1) == 0.0))
    ln2_trivial = bool(np.all(np.asarray(g2) == 1.0) and
                       np.all(np.asarray(be2) == 0.0))
    key = (ln1_trivial, ln2_trivial)
    if _cached is None or _cached[0] != key:
        _cached = (key, _build(ln1_trivial, ln2_trivial))
    nc = _cached[1]

    f16 = np.float16

    def shuf(w):
        # [C*128, out] -> [128, C*out] partition-major, fp16
        w = np.asarray(w, np.float32).astype(f16)
        cdim = w.shape[0] // 128
        return np.ascontiguousarray(
            w.reshape(cdim, 128, w.shape[1]).transpose(1, 0, 2).reshape(128, -1))

    weights = {name: shuf(arr) for name, arr in
               [("Wq", Wq), ("Wk", Wk), ("Wv", Wv), ("Wo", Wo), ("W2", W2)]}
    w1r = (np.asarray(W1, np.float32) * W1S).reshape(DC, 128, HID)
    if NC8:
        weights["W1a"] = np.ascontiguousarray(
            w1r[0:NC8].astype(ml_dtypes.float8_e4m3)
            .transpose(1, 0, 2).reshape(128, -1))
    weights["W1b"] = np.ascontiguousarray(
        w1r[NC8:].astype(f16).transpose(1, 0, 2).reshape(128, -1))
    bcb = np.stack([np.asarray(a, np.float32)
                    for a in [bv, bo, b2, g1, be1, g2, be2]]).astype(f16)
    weights["bcbr"] = np.ascontiguousarray(bcb.reshape(1, 7 * DIM))
    bpp = np.concatenate([
        np.asarray(bq, np.float32).reshape(DC, 128).T,
        np.asarray(bk, np.float32).reshape(DC, 128).T,
        np.asarray(b1, np.float32).reshape(HC, 128).T], axis=1)  # [128, 36]
    weights["bpp"] = np.ascontiguousarray(bpp)
    x = np.asarray(x, np.float32).astype(f16)

    in_maps = []
    for c in range(N_CORES):
        xc = np.ascontiguousarray(x[c * BPC:(c + 1) * BPC].reshape(T, DIM))
        # [768, T] -> [128, NPAIR, DC, PTP] pair-contiguous, flattened
        xt = xc.T
        xt4 = xt.reshape(DC, 128, NPAIR, PT).transpose(1, 2, 0, 3)
        xtp = np.zeros((128, NPAIR, DC, PTP), f16)
        xtp[:, :, :, :PT] = xt4
        xtc = np.ascontiguousarray(xtp.reshape(128, -1))
        in_maps.append({"x": xc, "xt": xtc, **weights})

    res = run_bass_kernel_spmd(nc, in_maps, core_ids=list(range(N_CORES)),
                               trace=bool(int(os.environ.get("BASSK_TRACE", "0"))))
    kernel._last_res = res
    out = np.concatenate(
        [res.results[c]["out"].reshape(BPC, S, DIM) for c in range(N_CORES)], axis=0)
    return out.astype(np.float32)


# revision 46
# speedup vs baseline: 1.0230x; 1.0230x over previous
"""Trainium2 Bass kernel for a ViT-Base transformer encoder block.

Input x: [64, 197, 768] fp32 + weights. Data-parallel over batch across 8
NeuronCores (8 batches/core = 1576 tokens/core). All matmul operands are
fp16 (fp32 PSUM accumulation): same PE throughput as bf16 but 8x finer
mantissa, so quantization error stays ~4.5e-4. Weights and x are cast to
fp16 host-side, and x is additionally passed pre-transposed (d-major) so no
PE transposes are needed in pass 1.

Per core, two passes over 4 batch-pairs (2 batches = 394 tokens each):

  pass 1: QKV projections, software-pipelined attention (per-batch
          197-col matmuls; odd heads write PSUM partitions 64:128 via
          tile_position; softmax denominators via rowsum matmuls +
          reciprocal_approx_fast + gpsimd partition_broadcast, two heads
          packed per broadcast tile so ctx eviction runs full-width),
          O-projection, LayerNorm1 + residual -> x1 kept in SBUF (fp16).
          Pair p+1's projections are interleaved into pair p's attention
          pipeline to keep the in-order PE queue dense.
  pass 2: MLP with W1/W2 resident in SBUF (fp16), exact GELU fused into
          the PSUM eviction, PE transpose back to token-major,
          LayerNorm2 + residual -> out.

LayerNorm scale/shift application runs on the vector engine (tensor_scalar
with per-partition rstd/-mu*rstd) so the scalar engine only ever runs Exp
(pass 1) / Gelu (pass 2) plus the LN Sqrt, minimizing activation-table
reloads. When gamma==1 / beta==0 (true for this problem's inputs, checked
at build time) the affine ops are folded into the residual add.
"""
import os
import sys

sys.path.insert(0, "/opt/trn_rl_repo")

import numpy as np
import ml_dtypes
from contextlib import ExitStack

import concourse.bass as bass
import concourse.tile as tile
from concourse import bacc, mybir
from concourse.bass_utils import run_bass_kernel_spmd
from concourse.masks import make_identity

DIM, NH, HD, HID = 768, 12, 64, 3072
S = 197
B = 64
N_CORES = 8
BPC = B // N_CORES            # 8 batches per core
T = BPC * S                   # 1576 tokens per core
NPAIR = BPC // 2              # 4 batch pairs per core
PT = 2 * S                    # 394 tokens per pair
PTP = 400                     # PT padded to a 16-elem multiple
EPS = 1e-6
DC = DIM // 128               # 6 d-chunks
HC = HID // 128               # 24 hidden chunks

F32 = mybir.dt.float32
F16 = mybir.dt.float16
FP8 = mybir.dt.float8e4
AF = mybir.ActivationFunctionType
OP = mybir.AluOpType
DR = mybir.MatmulPerfMode.DoubleRow

# Partial-fp8 MLP1: contract the first NC8 of 6 k-chunks in fp8 DoubleRow
# (2 chunks per PE instruction). Error budget: measured 5.4e-4 all-fp16;
# 4 fp8 chunks add ~1.8e-2 (sim), still under the 2e-2 gate.
NC8 = int(os.environ.get("BASSK_FP8_MLP1", "2"))
assert NC8 in (0, 2, 4)
W1S = 8.0 if NC8 else 1.0     # W1 pre-scale so fp8 chunks stay normal-range

# token tiles within a pair: (offset, size); tile 2*b + s for batch b
Q_TILES = [(0, 128), (128, 69), (197, 128), (325, 69)]

DEBUG = bool(int(os.environ.get("BASSK_DEBUG", "0")))

_cached = None


def _build(ln1_trivial, ln2_trivial):
    nc = bacc.Bacc("TRN2", target_bir_lowering=False, debug=False)

    # host-side pre-shuffled layouts: weights [128, C*out] fp16, xt
    # [128, NPAIR*DC*PTP] fp16 d-major, x [T, DIM] fp16 (residual path),
    # broadcast biases pre-tiled [128, 7*DIM], per-partition biases [128, 36]
    x_d = nc.dram_tensor("x", [T, DIM], F16, kind="ExternalInput").ap()
    xt_d = nc.dram_tensor("xt", [128, NPAIR * DC * PTP], F16,
                          kind="ExternalInput").ap()
    w_d = {}
    w1_specs = ([("W1a", [128, NC8 * HID], FP8),
                 ("W1b", [128, (DC - NC8) * HID], F16)] if NC8 else
                [("W1b", [128, DC * HID], F16)])
    for name, shape, dt in [("Wq", [128, DC * DIM], F16),
                            ("Wk", [128, DC * DIM], F16),
                            ("Wv", [128, DC * DIM], F16),
                            ("Wo", [128, DC * DIM], F16),
                            *w1_specs,
                            ("W2", [128, HC * DIM], F16),
                            ("bcbr", [1, 7 * DIM], F16),
                            ("bpp", [128, 36], F32)]:
        w_d[name] = nc.dram_tensor(name, shape, dt, kind="ExternalInput").ap()
    out_d = nc.dram_tensor("out", [T, DIM], F32, kind="ExternalOutput").ap()

    with tile.TileContext(nc) as tc, ExitStack() as octx:
        persist = octx.enter_context(tc.tile_pool(name="persist", bufs=1))

        # ---- pools for x slices (allocated before any DMA so the
        # startup-critical pair-0 transfers can be issued first) ----
        xt_view = xt_d.rearrange("p (r c t) -> p r c t", c=DC, t=PTP)

        # ---- constants / persistent tiles ----
        ident_f = persist.tile([128, 128], F32)
        ident_b = persist.tile([128, 128], F16)
        ones_b = persist.tile([128, 1], F16)
        eps_sb = persist.tile([128, 1], F32)
        bpp = persist.tile([128, 36], F32)
        bcb = persist.tile([128, 7, DIM], F16)
        Wt = {}
        for name in ["Wq", "Wk", "Wv", "Wo"]:
            Wt[name] = persist.tile([128, DC, DIM], F16, name=f"wt_{name}",
                                    tag=f"wt_{name}")
        W1a = (persist.tile([128, NC8, HID], FP8, name="w1a", tag="w1a")
               if NC8 else None)
        W1b = persist.tile([128, DC - NC8, HID], F16, name="w1b", tag="w1b")
        # x1 = attn block output, kept in SBUF across passes (fp16)
        x1_all = persist.tile([128, NPAIR, 4, DIM], F16)

        bq_sb = bpp[:, 0:DC]
        bk_sb = bpp[:, DC:2 * DC]
        b1_sb = bpp[:, 2 * DC:2 * DC + HC]
        bcast = {name: bcb[:, i, :]
                 for i, name in enumerate(
                     ["bv", "bo", "b2", "g1", "be1", "g2", "be2"])}

        # =========================== PASS 1 ===========================
        with ExitStack() as ctx:
            xtp_pool = ctx.enter_context(tc.tile_pool(name="xtf", bufs=2))
            xp = ctx.enter_context(tc.tile_pool(name="xp", bufs=3))
            big = ctx.enter_context(tc.tile_pool(name="p1big", bufs=2))
            exp_pool = ctx.enter_context(tc.tile_pool(name="exp", bufs=6))
            rt_pool = ctx.enter_context(tc.tile_pool(name="rt", bufs=2))
            bc_pool = ctx.enter_context(tc.tile_pool(name="bc", bufs=3))
            ao_pool = ctx.enter_context(tc.tile_pool(name="ao", bufs=2))
            ln_pool = ctx.enter_context(tc.tile_pool(name="ln", bufs=3))

            ps_mm = ctx.enter_context(tc.tile_pool(name="psmm", bufs=2, space="PSUM"))
            ps_sp = ctx.enter_context(tc.tile_pool(name="pssp", bufs=4, space="PSUM"))
            ps_cx = ctx.enter_context(tc.tile_pool(name="pscx", bufs=2, space="PSUM"))

            def load_xt(p):
                xT = xtp_pool.tile([128, DC, PTP], F16, tag="xT", name="xT")
                nc.sync.dma_start(xT[:], xt_view[:, p, :, :])
                return xT

            def load_x(p):
                g0 = p * PT
                x_sb = xp.tile([128, 4, DIM], F16, tag="x", name="x_sb")
                for i, (off, sz) in enumerate(Q_TILES):
                    nc.sync.dma_start(x_sb[0:sz, i, :], x_d[g0 + off:g0 + off + sz, :])
                return x_sb

            # --- startup-critical DMAs, most-urgent first per queue.
            # The first qk items need only the first 128-col block of Wq/Wk,
            # so those land first; W1 (pass-2) rides last on the gpsimd queue.
            xT_cur = load_xt(0)                      # sync queue
            wv = {n: w_d[n].rearrange("p (c j) -> p c j", j=DIM)
                  for n in ["Wq", "Wk", "Wv", "Wo"]}
            # weight loads spread over the scalar AND gpsimd DMA rings (a
            # single ring is the startup bottleneck at ~130GB/s); broadcast
            # biases come as one [1, 5376] row expanded on the gpsimd engine
            for c in range(DC):
                cb = slice(c * 128, (c + 1) * 128)
                nc.scalar.dma_start(Wt["Wq"][:, :, cb], wv["Wq"][:, :, cb])
                nc.gpsimd.dma_start(Wt["Wk"][:, :, cb], wv["Wk"][:, :, cb])
            nc.scalar.dma_start(Wt["Wv"][:], wv["Wv"])
            nc.sync.dma_start(bpp[:], w_d["bpp"])
            stg = ctx.enter_context(tc.tile_pool(name="stg", bufs=1))
            bcbr = stg.tile([128, 7 * DIM], F16, tag="bcbr", name="bcbr")
            nc.sync.dma_start(bcbr[0:1, :], w_d["bcbr"])
            nc.gpsimd.partition_broadcast(
                bcb.rearrange("p a j -> p (a j)"), bcbr[0:1, :], channels=128)
            make_identity(nc, ident_f[:])
            nc.vector.tensor_copy(ident_b[:], ident_f[:])
            nc.vector.memset(ones_b[:], 1.0)
            nc.vector.memset(eps_sb[:], EPS)
            xT_nxt = load_xt(1)
            nc.scalar.dma_start(Wt["Wo"][:], wv["Wo"])
            if NC8:
                nc.gpsimd.dma_start(
                    W1a[:], w_d["W1a"].rearrange("p (c j) -> p c j", j=HID))
            nc.gpsimd.dma_start(
                W1b[:], w_d["W1b"].rearrange("p (c j) -> p c j", j=HID))
            x_sb0 = load_x(0)
            nxt_x = load_x(1)

            # --- projection work items for one pair (list of thunks) ---
            def proj_items(p, xT, dst):
                qT = big.tile([128, DC, PT], F16, tag="qT", name="qT")
                kT = big.tile([128, DC, PT], F16, tag="kT", name="kT")
                v_sb = big.tile([128, 4, NH, HD], F16, tag="v", name="v_sb")
                dst["qT"], dst["kT"], dst["v_sb"] = qT, kT, v_sb
                items = []

                def qk_item(wname, bsb, dstT, c):
                    def run():
                        pm = ps_mm.tile([128, 512], F32, tag="mm", name="pm")
                        for kc in range(DC):
                            nc.tensor.matmul(pm[:, 0:PT],
                                             Wt[wname][:, kc, c * 128:(c + 1) * 128],
                                             xT[:, kc, 0:PT],
                                             start=(kc == 0), stop=(kc == DC - 1))
                        with nc.allow_low_precision(reason="qk f16"):
                            nc.vector.tensor_scalar(dstT[:, c, :], pm[:, 0:PT],
                                                    bsb[:, c:c + 1], None, OP.add)
                    return run

                def v_item(i, s):
                    def run():
                        off, sz = Q_TILES[i]
                        pm = ps_mm.tile([128, 512], F32, tag="mm", name="pm")
                        for kc in range(DC):
                            nc.tensor.matmul(pm[0:sz, 0:384],
                                             xT[:, kc, off:off + sz],
                                             Wt["Wv"][:, kc, s * 384:(s + 1) * 384],
                                             start=(kc == 0), stop=(kc == DC - 1))
                        with nc.allow_low_precision(reason="v f16"):
                            nc.vector.tensor_add(
                                v_sb[0:sz, i, 6 * s:6 * s + 6, :],
                                pm[0:sz, 0:384].rearrange("p (a b) -> p a b", a=6),
                                bcast["bv"][0:sz, s * 384:(s + 1) * 384]
                                    .rearrange("p (a b) -> p a b", a=6))
                    return run

                for c in range(DC):
                    items.append(qk_item("Wq", bq_sb, qT, c))
                    items.append(qk_item("Wk", bk_sb, kT, c))
                for i in range(4):
                    for s in range(2):
                        items.append(v_item(i, s))
                return items

            # --- attention for pair p, interleaving `items` (next pair's
            #     projections) between pipeline steps ---
            def attention(p, cur, items):
                qT, kT, v_sb = cur["qT"], cur["kT"], cur["v_sb"]
                ctxT = big.tile([128, DC, PTP], F16, tag="ctxT", name="ctxT")
                cur["ctxT"] = ctxT
                it = 0
                NSTEP = NH + 2
                exps = {}   # h -> [exp_s0, exp_s1]
                pcs = {}    # hc -> psum tile
                bcs = {}    # hc -> [128, PT] tile, head h in rows (h%2)*64
                for step in range(NSTEP):
                    # stage S: scores + exp for head `step`
                    if step < NH:
                        h = step
                        hc, hp = h // 2, (h % 2) * 64
                        exps[h] = []
                        for s in range(2):
                            ksz = Q_TILES[s][1]
                            psc = ps_sp.tile([128, 512], F32, tag="sp", name="psc")
                            for b in range(2):
                                koff = Q_TILES[2 * b + s][0]
                                cs = slice(b * S, (b + 1) * S)
                                nc.tensor.matmul(
                                    psc[0:ksz, cs],
                                    kT[hp:hp + 64, hc, koff:koff + ksz],
                                    qT[hp:hp + 64, hc, cs],
                                    start=True, stop=True,
                                    skip_group_check=True)
                            et = exp_pool.tile([128, 394], F16, tag="exp", name="et")
                            with nc.allow_low_precision(reason="softmax exp f16"):
                                nc.scalar.activation(et[0:ksz, :], psc[0:ksz, 0:PT],
                                                     AF.Exp, bias=0.0, scale=0.125)
                            exps[h].append(et)
                    # stage R: rowsum + recip + broadcast for head step-1
                    if 1 <= step <= NH:
                        h = step - 1
                        hc, hp = h // 2, (h % 2) * 64
                        pq = ps_sp.tile([128, 512], F32, tag="sp", name="pq")
                        for s in range(2):
                            ksz = Q_TILES[s][1]
                            nc.tensor.matmul(
                                pq[0:1, 0:PT],
                                ones_b[0:ksz, 0:1],
                                exps[h][s][0:ksz, :],
                                start=(s == 0), stop=(s == 1),
                                skip_group_check=True)
                        rt = rt_pool.tile([128, 394], F32, tag="rt", name="rt")
                        with nc.allow_low_precision(reason="softmax recip"):
                            nc.vector.reciprocal_approx_fast(
                                rt[0:1, :], pq[0:1, 0:PT])
                        # partition_broadcast can only target partition 0-based
                        # rows, so each head gets its own full broadcast tile
                        bcs[h] = bc_pool.tile([128, 394], F32, tag="bcsb",
                                              name="bc")
                        nc.gpsimd.partition_broadcast(bcs[h][:, :], rt[0:1, :],
                                                      channels=128)
                    # stage C: ctx for head step-1
                    if 1 <= step <= NH:
                        h = step - 1
                        hc, hp = h // 2, (h % 2) * 64
                        if hp == 0:
                            pcs[hc] = ps_cx.tile([128, 512], F32, tag="cx",
                                                 name=f"cx{hc}")
                        pc = pcs[hc]
                        for b in range(2):
                            cs = slice(b * S, (b + 1) * S)
                            for s in range(2):
                                ksz = Q_TILES[s][1]
                                nc.tensor.matmul(
                                    pc[hp:hp + 64, cs],
                                    v_sb[0:ksz, 2 * b + s, h, :],
                                    exps[h][s][0:ksz, cs],
                                    start=(s == 0), stop=(s == 1),
                                    skip_group_check=True)
                    # interleave next-pair projection work
                    for _ in range(1):
                        if it < len(items):
                            items[it]()
                            it += 1
                    # stage N: normalize head step-2
                    if step >= 2:
                        h2 = step - 2
                        hc, hp = h2 // 2, (h2 % 2) * 64
                        with nc.allow_low_precision(reason="ctx f16"):
                            nc.vector.tensor_tensor(
                                ctxT[hp:hp + 64, hc, 0:PT],
                                pcs[hc][hp:hp + 64, 0:PT],
                                bcs[h2][hp:hp + 64, :], OP.mult)
                return items[it:]

            def _ln_apply(dst, src, mv, eps_ap, gname, bname, trivial, res_ap,
                          sz):
                """LayerNorm scale/shift on the vector engine + residual add.

                src: [sz, DIM] f32 pre-LN values (mean/var in mv).
                dst: fp16 (or f32) destination; res_ap added in.
                """
                sd = ln_pool.tile([128, 2], F32, tag="sd", name="sd")
                nc.scalar.activation(sd[0:sz, 0:1], mv[0:sz, 1:2], AF.Sqrt,
                                     bias=eps_ap[0:sz, :], scale=1.0)
                rstd = ln_pool.tile([128, 1], F32, tag="rstd", name="rstd")
                with nc.allow_low_precision(reason="ln recip"):
                    nc.vector.reciprocal_approx_fast(rstd[0:sz, :], sd[0:sz, 0:1])
                nmr = ln_pool.tile([128, 1], F32, tag="nmr", name="nmr")
                nc.vector.tensor_scalar(nmr[0:sz, :], mv[0:sz, 0:1],
                                        rstd[0:sz, :], -1.0, OP.mult, OP.mult)
                tn = ao_pool.tile([128, DIM], F32, tag="tn", name="tn")
                nc.vector.tensor_scalar(tn[0:sz, :], src[0:sz, :],
                                        rstd[0:sz, :], nmr[0:sz, :],
                                        OP.mult, OP.add)
                with nc.allow_low_precision(reason="ln out"):
                    if trivial:
                        nc.vector.tensor_add(dst, tn[0:sz, :], res_ap)
                    else:
                        nc.vector.tensor_tensor(tn[0:sz, :], tn[0:sz, :],
                                                bcast[gname][0:sz, :], OP.mult)
                        nc.vector.tensor_add(tn[0:sz, :], tn[0:sz, :],
                                             bcast[bname][0:sz, :])
                        nc.vector.tensor_add(dst, tn[0:sz, :], res_ap)

            def o_proj_ln1(p, cur, items):
                it = 0
                ctxT, x_sb = cur["ctxT"], cur["x_sb"]
                for i, (off, sz) in enumerate(Q_TILES):
                    ao = ao_pool.tile([128, DIM], F32, tag="ao", name="ao")
                    for s in range(2):
                        pm = ps_mm.tile([128, 512], F32, tag="mm", name="pm")
                        for c in range(DC):
                            nc.tensor.matmul(pm[0:sz, 0:384],
                                             ctxT[:, c, off:off + sz],
                                             Wt["Wo"][:, c, s * 384:(s + 1) * 384],
                                             start=(c == 0), stop=(c == DC - 1))
                        nc.vector.tensor_add(ao[0:sz, s * 384:(s + 1) * 384],
                                             pm[0:sz, 0:384],
                                             bcast["bo"][0:sz, s * 384:(s + 1) * 384])
                        if it < len(items):
                            items[it]()
                            it += 1
                    # LayerNorm 1 (stats on vector, 2 groups of 384)
                    st = ln_pool.tile([128, 2, nc.vector.BN_STATS_DIM], F32,
                                      tag="st", name="st")
                    for g in range(2):
                        nc.vector.bn_stats(st[0:sz, g, :],
                                           ao[0:sz, g * 384:(g + 1) * 384])
                    mv = ln_pool.tile([128, nc.vector.BN_AGGR_DIM], F32, tag="mv",
                                      name="mv")
                    nc.vector.bn_aggr(mv[0:sz, :], st[0:sz, :, :])
                    _ln_apply(x1_all[0:sz, p, i, :], ao, mv, eps_sb,
                              "g1", "be1", ln1_trivial, x_sb[0:sz, i, :], sz)
                while it < len(items):
                    items[it]()
                    it += 1

            # ---- pass-1 driver: pipelined over pairs ----
            cur = {}
            cur["x_sb"] = x_sb0
            for item in proj_items(0, xT_cur, cur):
                item()
            for p in range(NPAIR):
                if p + 1 < NPAIR:
                    nxt = {"x_sb": nxt_x}
                    items = proj_items(p + 1, xT_nxt, nxt)
                else:
                    nxt = None
                    items = []
                if p + 2 < NPAIR:
                    xT_nxt2 = load_xt(p + 2)
                left = attention(p, cur, items)
                if p + 2 < NPAIR:
                    nxt_x = load_x(p + 2)
                o_proj_ln1(p, cur, left)
                cur = nxt
                if p + 2 < NPAIR:
                    xT_cur, xT_nxt = xT_nxt, xT_nxt2

        # =========================== PASS 2 ===========================
        with ExitStack() as ctx:
            wpool = ctx.enter_context(tc.tile_pool(name="w2p", bufs=1))
            W2t = wpool.tile([128, HC, DIM], F16)
            # split so the first chunks (needed at hcx=LAG) land early
            w2v = w_d["W2"].rearrange("p (c j) -> p c j", j=DIM)
            nc.sync.dma_start(W2t[:, 0:6, :], w2v[:, 0:6, :])
            nc.sync.dma_start(W2t[:, 6:HC, :], w2v[:, 6:HC, :])

            xtp = ctx.enter_context(tc.tile_pool(name="xtp", bufs=2))
            htp = ctx.enter_context(tc.tile_pool(name="htp", bufs=1))
            mo_pool = ctx.enter_context(tc.tile_pool(name="mo", bufs=2))
            moT_pool = ctx.enter_context(tc.tile_pool(name="moT", bufs=2))
            ln_pool = ctx.enter_context(tc.tile_pool(name="ln2", bufs=3))
            ao2_pool = ctx.enter_context(tc.tile_pool(name="ao2", bufs=2))
            out_pool = ctx.enter_context(tc.tile_pool(name="outp", bufs=2))

            ps_wk = ctx.enter_context(tc.tile_pool(name="pswk", bufs=2, space="PSUM"))
            ps_ac = ctx.enter_context(tc.tile_pool(name="psac", bufs=6, space="PSUM"))

            def x1t_items(p, box):
                if NC8:
                    x1T8 = xtp.tile([128, NC8, PTP], FP8, tag="x1T8",
                                    name="x1T8")
                    box["x1T8"] = x1T8
                x1T = xtp.tile([128, DC - NC8, PTP], F16, tag="x1T", name="x1T")
                box["x1T"] = x1T

                def one(i):
                    def run():
                        off, sz = Q_TILES[i]
                        # 6 transposes into one PSUM bank (f16 view), then a
                        # single strided eviction per destination dtype
                        pt = ps_wk.tile([128, 512], F32, tag="wk", name="pt")
                        ptb = pt[:, 0:384].bitcast(F16).rearrange(
                            "p (c t) -> p c t", c=DC)
                        for c in range(DC):
                            nc.tensor.transpose(
                                ptb[:, c, 0:sz],
                                x1_all[0:sz, p, i, c * 128:(c + 1) * 128],
                                ident_b[0:sz, 0:sz])
                        with nc.allow_low_precision(reason="x1T f16/fp8"):
                            if NC8:
                                nc.vector.tensor_copy(x1T8[:, :, off:off + sz],
                                                      ptb[:, 0:NC8, 0:sz])
                            nc.vector.tensor_copy(x1T[:, :, off:off + sz],
                                                  ptb[:, NC8:DC, 0:sz])
                    return run
                return [one(i) for i in range(4)]

            LAG = 6

            def mlp(p, xbox, evict_items, tail_its, mcols=(0, PT)):
                x1T = xbox["x1T"]
                x1T8 = xbox.get("x1T8")
                hT = htp.tile([128, HC, PTP], F16, tag="hT", name="hT")
                pacs = [ps_ac.tile([128, 512], F32, tag="ac", name=f"pac{c}")
                        for c in range(DC)]
                m0, m1 = mcols
                ti = 0
                for hcx in range(HC + LAG):
                    if hcx < HC:
                        pm = ps_wk.tile([128, 512], F32, tag="wk", name="pm")
                        hs = slice(hcx * 128, (hcx + 1) * 128)
                        for kc in range(0, NC8, 2):
                            nc.tensor.matmul(pm[:, 0:PT],
                                             W1a[:, kc:kc + 2, hs],
                                             x1T8[:, kc:kc + 2, 0:PT],
                                             start=(kc == 0), stop=False,
                                             perf_mode=DR,
                                             skip_group_check=True)
                        for j in range(DC - NC8):
                            nc.tensor.matmul(pm[:, 0:PT],
                                             W1b[:, j, hs],
                                             x1T[:, j, 0:PT],
                                             start=(NC8 == 0 and j == 0),
                                             stop=(j == DC - NC8 - 1),
                                             skip_group_check=True)
                        with nc.allow_low_precision(reason="h f16"):
                            nc.scalar.activation(hT[:, hcx, 0:PT], pm[:, 0:PT],
                                                 AF.Gelu,
                                                 bias=b1_sb[:, hcx:hcx + 1],
                                                 scale=1.0 / W1S)
                        if hcx < len(evict_items):
                            evict_items[hcx]()
                    h2 = hcx - LAG
                    if h2 >= 0:
                        for c in range(DC):
                            nc.tensor.matmul(pacs[c][:, m0:m1],
                                             W2t[:, h2, c * 128:(c + 1) * 128],
                                             hT[:, h2, m0:m1],
                                             start=(h2 == 0), stop=(h2 == HC - 1))
                    if hcx >= LAG and hcx % 2 == 0 and ti < len(tail_its):
                        tail_its[ti]()
                        ti += 1
                while ti < len(tail_its):
                    tail_its[ti]()
                    ti += 1
                return pacs, hT

            def evict_items_for(p, pacs, box, cols=(0, PT)):
                if "moT" not in box:
                    box["moT"] = moT_pool.tile([128, DC, PTP], F16, tag="moT",
                                               name="moT")
                moT = box["moT"]
                c0, c1 = cols

                def one(c):
                    def run():
                        with nc.allow_low_precision(reason="moT f16"):
                            nc.vector.tensor_copy(moT[:, c, c0:c1],
                                                  pacs[c][:, c0:c1])
                    return run
                return [one(c) for c in range(DC)]

            def tail_items_for(p, box):
                g0 = p * PT
                moT = box["moT"]

                def one(i):
                    def run():
                        off, sz = Q_TILES[i]
                        _ln2_tile(p, g0, moT, i, off, sz)
                    return run
                return [one(i) for i in range(4)]

            def _ln2_tile(p, g0, moT, i, off, sz):
                mo = mo_pool.tile([128, DIM], F32, tag="mo", name="mo")
                pt = ps_wk.tile([128, 512], F32, tag="wk", name="pt")
                ptb = pt[:, 0:384].bitcast(F16).rearrange(
                    "p (c t) -> p c t", c=DC)
                for c in range(DC):
                    nc.tensor.transpose(ptb[0:sz, c, 0:128],
                                        moT[:, c, off:off + sz], ident_b[:, :])
                nc.vector.tensor_add(
                    mo[0:sz, :],
                    ptb[0:sz, :, :].rearrange("p c t -> p (c t)"),
                    bcast["b2"][0:sz, :])
                # LayerNorm 2 + residual
                st = ln_pool.tile([128, 2, nc.vector.BN_STATS_DIM], F32,
                                  tag="st", name="st")
                for g in range(2):
                    nc.vector.bn_stats(st[0:sz, g, :],
                                       mo[0:sz, g * 384:(g + 1) * 384])
                mv = ln_pool.tile([128, nc.vector.BN_AGGR_DIM], F32, tag="mv",
                                  name="mv")
                nc.vector.bn_aggr(mv[0:sz, :], st[0:sz, :, :])
                sd = ln_pool.tile([128, 2], F32, tag="sd", name="sd")
                nc.scalar.activation(sd[0:sz, 0:1], mv[0:sz, 1:2], AF.Sqrt,
                                     bias=eps_sb[0:sz, :], scale=1.0)
                rstd = ln_pool.tile([128, 1], F32, tag="rstd", name="rstd")
                with nc.allow_low_precision(reason="ln2 recip"):
                    nc.vector.reciprocal_approx_fast(rstd[0:sz, :], sd[0:sz, 0:1])
                nmr = ln_pool.tile([128, 1], F32, tag="nmr", name="nmr")
                nc.vector.tensor_scalar(nmr[0:sz, :], mv[0:sz, 0:1],
                                        rstd[0:sz, :], -1.0, OP.mult, OP.mult)
                tln = ao2_pool.tile([128, DIM], F32, tag="tln", name="tln")
                nc.vector.tensor_scalar(tln[0:sz, :], mo[0:sz, :],
                                        rstd[0:sz, :], nmr[0:sz, :],
                                        OP.mult, OP.add)
                ot = out_pool.tile([128, DIM], F32, tag="ot", name="ot")
                if ln2_trivial:
                    nc.vector.tensor_add(ot[0:sz, :], tln[0:sz, :],
                                         x1_all[0:sz, p, i, :])
                else:
                    nc.vector.tensor_tensor(tln[0:sz, :], tln[0:sz, :],
                                            bcast["g2"][0:sz, :], OP.mult)
                    nc.vector.tensor_add(tln[0:sz, :], tln[0:sz, :],
                                         bcast["be2"][0:sz, :])
                    nc.vector.tensor_add(ot[0:sz, :], tln[0:sz, :],
                                         x1_all[0:sz, p, i, :])
                nc.sync.dma_start(out_d[g0 + off:g0 + off + sz, :], ot[0:sz, :])

            box0 = {}
            for it in x1t_items(0, box0):
                it()
            xbox_cur = box0
            prev_pacs = prev_hT = None
            for p in range(NPAIR):
                ev = []
                tl = []
                if p > 0:
                    pbox = {}
                    ev = evict_items_for(p - 1, prev_pacs, pbox)
                    tl = tail_items_for(p - 1, pbox)
                nbox = {}
                nxt_items = x1t_items(p + 1, nbox) if p + 1 < NPAIR else []
                last = p == NPAIR - 1
                prev_pacs, prev_hT = mlp(p, xbox_cur, ev, tl + nxt_items,
                                         mcols=(0, 128) if last else (0, PT))
                if p + 1 < NPAIR:
                    xbox_cur = nbox
            # final tail: the last pair's MLP2 runs per token tile, each
            # tile's evictions + LN2 hidden under the next tile's matmuls
            fbox = {}
            pend = evict_items_for(NPAIR - 1, prev_pacs, fbox, cols=(0, 128))
            tl_items = tail_items_for(NPAIR - 1, fbox)
            pend = pend + [tl_items[0]]
            for tix in range(1, 4):
                off, sz = Q_TILES[tix]
                ai = 0
                for h2 in range(HC):
                    for c in range(DC):
                        nc.tensor.matmul(prev_pacs[c][:, off:off + sz],
                                         W2t[:, h2, c * 128:(c + 1) * 128],
                                         prev_hT[:, h2, off:off + sz],
                                         start=(h2 == 0), stop=(h2 == HC - 1))
                    if h2 >= 1 and h2 % 3 == 1 and ai < len(pend):
                        pend[ai]()
                        ai += 1
                while ai < len(pend):
                    pend[ai]()
                    ai += 1
                pend = (evict_items_for(NPAIR - 1, prev_pacs, fbox,
                                        cols=(off, off + sz)) +
                        [tl_items[tix]])
            for it in pend:
                it()

    nc.compile()
    return nc


def kernel(x, Wq, bq, Wk, bk, Wv, bv, Wo, bo, W1, b1, W2, b2, g1, be1, g2, be2):
    global _cached
    ln1_trivial = bool(np.all(np.asarray(g1) == 1.0) and
                       np.all(np.asarray(be1) == 0.0))
    ln2_trivial = bool(np.all(np.asarray(g2) == 1.0) and
                       np.all(np.asarray(be2) == 0.0))
    key = (ln1_trivial, ln2_trivial)
    if _cached is None or _cached[0] != key:
        _cached = (key, _build(ln1_trivial, ln2_trivial))
    nc = _cached[1]

    f16 = np.float16

    def shuf(w):
        # [C*128, out] -> [128, C*out] partition-major, fp16
        w = np.asarray(w, np.float32).astype(f16)
        cdim = w.shape[0] // 128
        return np.ascontiguousarray(
            w.reshape(cdim, 128, w.shape[1]).transpose(1, 0, 2).reshape(128, -1))

    weights = {name: shuf(arr) for name, arr in
               [("Wq", Wq), ("Wk", Wk), ("Wv", Wv), ("Wo", Wo), ("W2", W2)]}
    w1r = (np.asarray(W1, np.float32) * W1S).reshape(DC, 128, HID)
    if NC8:
        weights["W1a"] = np.ascontiguousarray(
            w1r[0:NC8].astype(ml_dtypes.float8_e4m3)
            .transpose(1, 0, 2).reshape(128, -1))
    weights["W1b"] = np.ascontiguousarray(
        w1r[NC8:].astype(f16).transpose(1, 0, 2).reshape(128, -1))
    bcb = np.stack([np.asarray(a, np.float32)
                    for a in [bv, bo, b2, g1, be1, g2, be2]]).astype(f16)
    weights["bcbr"] = np.ascontiguousarray(bcb.reshape(1, 7 * DIM))
    bpp = np.concatenate([
        np.asarray(bq, np.float32).reshape(DC, 128).T,
        np.asarray(bk, np.float32).reshape(DC, 128).T,
        np.asarray(b1, np.float32).reshape(HC, 128).T], axis=1)  # [128, 36]
    weights["bpp"] = np.ascontiguousarray(bpp)
    x = np.asarray(x, np.float32).astype(f16)

    in_maps = []
    for c in range(N_CORES):
        xc = np.ascontiguousarray(x[c * BPC:(c + 1) * BPC].reshape(T, DIM))
        # [768, T] -> [128, NPAIR, DC, PTP] pair-contiguous, flattened
        xt = xc.T
        xt4 = xt.reshape(DC, 128, NPAIR, PT).transpose(1, 2, 0, 3)
        xtp = np.zeros((128, NPAIR, DC, PTP), f16)
        xtp[:, :, :, :PT] = xt4
        xtc = np.ascontiguousarray(xtp.reshape(128, -1))
        in_maps.append({"x": xc, "xt": xtc, **weights})

    res = run_bass_kernel_spmd(nc, in_maps, core_ids=list(range(N_CORES)),
                               trace=bool(int(os.environ.get("BASSK_TRACE", "0"))))
    kernel._last_res = res
    out = np.concatenate(
        [res.results[c]["out"].reshape(BPC, S, DIM) for c in range(N_CORES)], axis=0)
    return out.astype(np.float32)
